# revision 3
# baseline (speedup 1.0000x reference)
"""Trainium2 Bass kernel for a transformer block (LN1->MHA->+res->LN2->FFN->+res).

Sharding: data-parallel over batch. B=8 batch elements == 8 NeuronCores; each
core runs the whole block for one batch element (no collectives).

Per-core dataflow (T=1024, D=1024, 16 heads x 64), tuned for PE clock (HAM)
warmth -- the PE runs at 1.2 GHz until it has been busy for a full ~3.4us
window, then 2.4 GHz; every phase keeps a dense back-to-back matmul stream:
  - ~9 dummy warmup matmuls at kernel start flip the HAM throttle to 8/8
    before the first real QKV matmul
  - LN1 token-major per 128-token chunk (ACT accum_out stats); gamma/beta
    folded into the transpose-evacuation tensor_scalar; activations then
    FEATURE-major [D, T] (f32r) through the dense chain
  - causal attention per head: scoresT[s,t] block-rows land in a single
    2-bank PSUM tile [128,1024]; ONE Exp ACT per (head, s-block) covers the
    whole causal range (fuses 1/8 scale + f32->bf16); score matmuls are
    causally tightened; diagonal masked post-exp on GpSimd
  - softmax denominator from a ones-column in v (attn@[v|1]); per-partition
    reciprocal + tensor_scalar divide on DVE
  - projection in bf16 (weights streamed bf16); LN2 stats (ones-matmuls)
    interleaved with the projection matmuls, row sums copied to SBUF at proj
    end; LN2 row ops + per-feature apply feed both an f32 token-major
    residual ledger (xp2, +b2) and a bf16 copy (xb2) for ff1
  - FFN: w1/w2 streamed bf16 (w1 first set prefetched during attention,
    rotating through a 10-deep pool); relu+bias evac on the (idle) Scalar
    engine; ff2 partial sums accumulate straight into xp2 on DVE
"""

import sys

sys.path.insert(0, "/opt/trn_rl_repo")

import numpy as np
import ml_dtypes

import concourse.bacc as bacc
import concourse.mybir as mybir
from concourse.tile import TileContext
from concourse import bass_utils

F32 = mybir.dt.float32
F32R = mybir.dt.float32r
BF16 = mybir.dt.bfloat16
AF = mybir.ActivationFunctionType
ALU = mybir.AluOpType

B, T, D = 8, 1024, 1024
H, E = 16, 64
DFF = 4 * D
NCORE = 8
NT = T // 128
ND = D // 128
NJ = DFF // 128
LN_EPS = 1e-5


class _Done(Exception):
    pass


def r32(ap):
    return ap.bitcast(F32R)


def v32(ap):
    return ap.bitcast(F32)


def _build(upto=9):
    nc = bacc.Bacc("TRN2", target_bir_lowering=False, debug=False,
                   num_devices=NCORE)

    x_l = nc.dram_tensor("x_l", [T, D], F32R, kind="ExternalInput")
    wqp = nc.dram_tensor("wqp", [H // 2, 128, ND, 128], F32R,
                         kind="ExternalInput")
    wkp = nc.dram_tensor("wkp", [H // 2, 128, ND, 128], F32R,
                         kind="ExternalInput")
    wva = nc.dram_tensor("wva", [D, D], F32R, kind="ExternalInput")
    wpj = nc.dram_tensor("wpj", [D, D], BF16, kind="ExternalInput")
    w1r = nc.dram_tensor("w1r", [NJ, 128, ND, 128], BF16,
                         kind="ExternalInput")
    w2b = nc.dram_tensor("w2b", [DFF, D], BF16, kind="ExternalInput")
    g1f = nc.dram_tensor("g1f", [128, ND], F32, kind="ExternalInput")
    be1f = nc.dram_tensor("be1f", [128, ND], F32, kind="ExternalInput")
    g2f = nc.dram_tensor("g2f", [128, ND], F32, kind="ExternalInput")
    be2f = nc.dram_tensor("be2f", [128, ND], F32, kind="ExternalInput")
    bpf = nc.dram_tensor("bpf", [128, ND], F32, kind="ExternalInput")
    b1f = nc.dram_tensor("b1f", [128, NJ], F32, kind="ExternalInput")
    b2b = nc.dram_tensor("b2b", [128, D], F32, kind="ExternalInput")
    idn = nc.dram_tensor("idn", [128, 128], F32R, kind="ExternalInput")
    onz = nc.dram_tensor("onz", [128, 128], F32R, kind="ExternalInput")
    ond = nc.dram_tensor("ond", [128, 1], F32R, kind="ExternalInput")
    mby = nc.dram_tensor("mby", [128, 128], BF16, kind="ExternalInput")
    out_l = nc.dram_tensor("out_l", [T, D], F32, kind="ExternalOutput")

    def dump_fm(tiles, n):
        for c in range(n):
            nc.sync.dma_start(out_l[128 * c:128 * (c + 1), :],
                              v32(tiles[c][:]))

    with TileContext(nc) as tc:
        with tc.tile_pool(name="const", bufs=1) as cp:
          try:
            def cload(name, dram, shape, dtype=F32):
                t = cp.tile(list(shape), dtype, tag=name, name=name)
                nc.sync.dma_start(t[:], dram[:])
                return t

            c_id = cload("idn", idn, [128, 128], F32R)
            c_g1 = cload("g1f", g1f, [128, ND])
            c_be1 = cload("be1f", be1f, [128, ND])
            c_eps = cp.tile([128, 1], F32, tag="eps", name="eps")
            nc.gpsimd.memset(c_eps[:], LN_EPS)
            c_wrm = cp.tile([128, 512], F32, tag="wrm", name="wrm")
            nc.gpsimd.memset(c_wrm[:], 0.0)

            # ---- HAM warmup: ~4us of dummy matmuls so the PE clock is at
            # 2.4 GHz when the first real matmul issues ----
            with tc.tile_pool(name="ps_wrm", bufs=1, space="PSUM") as psw:
                for _ in range(18):
                    pw = psw.tile([128, 512], F32, tag="wrm", bufs=2,
                                  name="pwrm")
                    nc.tensor.matmul(pw[:], c_id[:], r32(c_wrm[:]),
                                     start=True, stop=True)

            with (
                tc.tile_pool(name="w1s", bufs=10) as w1sp,
                tc.tile_pool(name="fmx", bufs=8) as fmx,
                tc.tile_pool(name="rw2", bufs=1) as rw2,
            ):
                # ========== phase 1: LN1 (token-major) ==========
                xf = [fmx.tile([128, T], F32R, tag="xf", name="xf")
                      for _ in range(ND)]
                with (
                    tc.tile_pool(name="xin", bufs=3) as xp,
                    tc.tile_pool(name="scr", bufs=2) as scrp,
                    tc.tile_pool(name="st1", bufs=4) as st1,
                    tc.tile_pool(name="ps_a", bufs=1, space="PSUM") as psa,
                ):
                    for m in range(NT):
                        xm = xp.tile([128, D], F32R, tag="xin")
                        nc.sync.dma_start(xm[:], x_l[128 * m:128 * (m + 1), :])
                        scr = scrp.tile([128, D], F32, tag="scr", name="scr")
                        st = st1.tile([128, 4], F32, tag="st", name="st")
                        nc.scalar.activation(scr[:], xm[:], AF.Square,
                                             accum_out=st[:, 1:2])
                        nc.scalar.activation(scr[:], xm[:], AF.Identity,
                                             accum_out=st[:, 0:1])
                        nc.vector.tensor_scalar_mul(st[:, 0:1], st[:, 0:1],
                                                    1.0 / D)
                        nc.vector.tensor_scalar_mul(st[:, 1:2], st[:, 1:2],
                                                    1.0 / D)
                        nc.vector.tensor_mul(st[:, 2:3], st[:, 0:1],
                                             st[:, 0:1])
                        nc.vector.tensor_sub(st[:, 2:3], st[:, 1:2],
                                             st[:, 2:3])
                        nc.scalar.activation(st[:, 2:3], st[:, 2:3], AF.Sqrt,
                                             bias=c_eps[:])
                        nc.vector.reciprocal(st[:, 3:4], st[:, 2:3])
                        nc.vector.tensor_scalar(
                            out=xm[:], in0=xm[:], scalar1=st[:, 0:1],
                            scalar2=st[:, 3:4], op0=ALU.subtract, op1=ALU.mult)
                        for c in range(ND):
                            pt = psa.tile([128, 128], F32R, tag="tr",
                                          bufs=3, name="pt")
                            nc.tensor.transpose(
                                pt[:], xm[:, 128 * c:128 * (c + 1)], c_id[:])
                            nc.vector.tensor_scalar(
                                out=xf[c][:, 128 * m:128 * (m + 1)],
                                in0=pt[:], scalar1=c_g1[:, c:c + 1],
                                scalar2=c_be1[:, c:c + 1],
                                op0=ALU.mult, op1=ALU.add)
                # non-critical consts load behind the x chunks
                c_idb = cp.tile([128, 128], BF16, tag="idnb", name="idnb")
                nc.vector.tensor_copy(c_idb[:], c_id[:])
                c_on = cload("onz", onz, [128, 128], F32R)
                c_od = cload("ond", ond, [128, 1], F32R)
                c_g2 = cload("g2f", g2f, [128, ND])
                c_be2 = cload("be2f", be2f, [128, ND])
                c_bp = cload("bpf", bpf, [128, ND])
                c_b1 = cload("b1f", b1f, [128, NJ])
                c_b2 = cload("b2b", b2b, [128, D])
                c_mb = cload("mby", mby, [128, 128], BF16)
                if upto == 1:
                    dump_fm(xf, ND)
                    raise _Done()
                xln1 = xf

                # prefetch FFN w1 set 0 (runs during attention on idle DMA)
                w1q = []
                for j8 in range(8):
                    t = w1sp.tile([128, D], BF16, tag="w1", name="w1t")
                    nc.sync.dma_start(
                        t[:], w1r[j8].rearrange("a b c -> a (b c)"))
                    w1q.append(t)

                # LN2 row accumulators (written at proj end, read in phase 5)
                mu_r = rw2.tile([1, T], F32, tag="mu_r")
                ms_r = rw2.tile([1, T], F32, tag="ms_r")

                with tc.tile_pool(name="fma", bufs=8) as fma:
                  aoT = [fma.tile([128, T], BF16, tag="ao", name="aoT")
                         for _ in range(ND)]
                  with (
                      tc.tile_pool(name="qk", bufs=16) as qkp,
                      tc.tile_pool(name="vsb", bufs=8) as vp,
                  ):
                    # ========== phase 2: QKV + v ==========
                    qT, kT, v_sb = [], [], []
                    with (
                        tc.tile_pool(name="wq", bufs=2) as wqpool,
                        tc.tile_pool(name="wk", bufs=2) as wkpool,
                        tc.tile_pool(name="ps_q", bufs=1, space="PSUM") as psq,
                    ):
                        for p in range(H // 2):
                            wq_t = wqpool.tile([128, D], F32R, tag="wq")
                            nc.sync.dma_start(
                                wq_t[:], wqp[p].rearrange("a b c -> a (b c)"))
                            wk_t = wkpool.tile([128, D], F32R, tag="wk")
                            nc.sync.dma_start(
                                wk_t[:], wkp[p].rearrange("a b c -> a (b c)"))
                            q_t = qkp.tile([128, T], BF16, tag="qk")
                            k_t = qkp.tile([128, T], BF16, tag="qk")
                            for dst, w_t in ((q_t, wq_t), (k_t, wk_t)):
                                for jj in range(2):
                                    sl = slice(512 * jj, 512 * (jj + 1))
                                    ps = psq.tile([128, 512], F32, tag="qk",
                                                  bufs=3, name="psqk")
                                    for c in range(ND):
                                        nc.tensor.matmul(
                                            ps[:],
                                            w_t[:, 128 * c:128 * (c + 1)],
                                            xln1[c][:, sl],
                                            start=(c == 0),
                                            stop=(c == ND - 1))
                                    nc.vector.tensor_copy(dst[:, sl], ps[:])
                            qT.append(q_t)
                            kT.append(k_t)
                        with tc.tile_pool(name="wv", bufs=8) as wvpool:
                            wv_t = []
                            for c in range(ND):
                                w = wvpool.tile([128, D], F32R, tag="wv",
                                                name="wv_t")
                                nc.sync.dma_start(
                                    w[:], wva[128 * c:128 * (c + 1), :])
                                wv_t.append(w)
                            for i in range(NT):
                                vt = vp.tile([128, H * 65], BF16, tag="v",
                                             name="vt")
                                v3 = vt.rearrange("p (h e) -> p h e", e=65)
                                nc.gpsimd.memset(v3[:, :, 64:65], 1.0)
                                for nb in range(2):
                                    ps = psq.tile([128, 512], F32, tag="v",
                                                  bufs=2, name="psv")
                                    for c in range(ND):
                                        nc.tensor.matmul(
                                            ps[:],
                                            xln1[c][:, 128 * i:128 * (i + 1)],
                                            wv_t[c][:, 512 * nb:512 * (nb + 1)],
                                            start=(c == 0), stop=(c == ND - 1))
                                    nc.scalar.copy(
                                        v3[:, 8 * nb:8 * (nb + 1), 0:64],
                                        ps[:].rearrange("p (h e) -> p h e",
                                                        e=64))
                                v_sb.append(vt)
                    if upto == 2:
                        for c in range(2):
                            nc.sync.dma_start(
                                out_l[128 * c:128 * (c + 1), :],
                                v32(qT[c][:]))
                            nc.sync.dma_start(
                                out_l[128 * (c + 2):128 * (c + 3), :],
                                v32(kT[c][:]))
                        raise _Done()

                    # ===== phase 3: causal attention (v-stationary av) =====
                    # av flipped: out[e,t] = v^T @ se accumulated over key
                    # blocks i straight into a [65,T] psum (row 64 = softmax
                    # denominator via the ones-column in v).  One matmul per
                    # (i, psum bank) instead of per (i, token block): large
                    # moving operands, and the output lands feature-major so
                    # the proj transposes disappear.  Odd heads reach
                    # partitions 64:128 of the pair tile via SBUF->SBUF DMA
                    # (DVE cannot shift partitions).
                    with (
                        tc.tile_pool(name="sc", bufs=12) as scp,
                        tc.tile_pool(name="dvt", bufs=1) as dvp,
                        tc.tile_pool(name="ps_b", bufs=1, space="PSUM") as psb,
                    ):
                        v3i = [v_sb[i].rearrange("p (h e) -> p h e", e=65)
                               for i in range(NT)]
                        Us = [None] * H

                        def divide_head(h):
                            # softmax divide for head h, one head behind the
                            # main loop so the recip->broadcast->mul chain
                            # never blocks the next head's mask/exp stream
                            p, q = h // 2, h % 2
                            U = Us[h]
                            rrow = dvp.tile([1, T], F32, tag="rrow",
                                            bufs=2, name="rrow")
                            nc.vector.reciprocal(rrow[:], U[64:65, :])
                            rb = dvp.tile([64, T], F32, tag="rb",
                                          bufs=2, name="rb")
                            nc.gpsimd.partition_broadcast(rb[:],
                                                          rrow[0:1, :])
                            if q == 0:
                                nc.vector.tensor_mul(
                                    aoT[p][0:64, :], U[0:64, :], rb[:])
                            else:
                                tmp = dvp.tile([64, T], BF16, tag="tmp",
                                               bufs=2, name="tmp")
                                nc.vector.tensor_mul(
                                    tmp[:], U[0:64, :], rb[:])
                                nc.sync.dma_start(aoT[p][64:128, :],
                                                  tmp[:])

                        for h in range(H):
                            p, q = h // 2, h % 2
                            qsl = slice(64 * q, 64 * (q + 1))
                            if h > 0:
                                rrow = dvp.tile([1, T], F32, tag="rrow",
                                                bufs=2, name="rrow")
                                nc.vector.reciprocal(rrow[:],
                                                     Us[h - 1][64:65, :])
                            se = []
                            for i in range(NT):
                                st = scp.tile([128, T], BF16, tag="sc",
                                              name="se")
                                sc2 = psb.tile([128, T], F32, tag="sc2",
                                               bufs=2, name="sc2")
                                kblk = kT[p][qsl, 128 * i:128 * (i + 1)]
                                if i < 4:
                                    nc.tensor.matmul(
                                        sc2[:, 128 * i:512], kblk,
                                        qT[p][qsl, 128 * i:512],
                                        start=True, stop=True)
                                lo2 = max(512, 128 * i)
                                nc.tensor.matmul(
                                    sc2[:, lo2:1024], kblk,
                                    qT[p][qsl, lo2:1024],
                                    start=True, stop=True)
                                nc.scalar.activation(
                                    st[:, 128 * i:1024],
                                    sc2[:, 128 * i:1024],
                                    AF.Exp, scale=0.125)
                                dg = slice(128 * i, 128 * (i + 1))
                                nc.gpsimd.tensor_mul(
                                    st[:, dg], st[:, dg], c_mb[:])
                                se.append(st)
                            U = psb.tile([65, T], F32, tag="U", bufs=2,
                                         name="U")
                            Us[h] = U
                            for i in range(NT):
                                lo = 128 * i
                                if lo < 512:
                                    nc.tensor.matmul(
                                        U[:, lo:512], v3i[i][:, h, :],
                                        se[i][:, lo:512],
                                        start=(i == 0), stop=(i == 3),
                                        skip_group_check=True)
                                lo2 = max(lo, 512)
                                nc.tensor.matmul(
                                    U[:, lo2:1024], v3i[i][:, h, :],
                                    se[i][:, lo2:1024],
                                    start=(i == 0), stop=(i == NT - 1),
                                    skip_group_check=True)
                            if h > 0:
                                ph, qh = (h - 1) // 2, (h - 1) % 2
                                rb = dvp.tile([64, T], F32, tag="rb",
                                              bufs=2, name="rb")
                                nc.gpsimd.partition_broadcast(
                                    rb[:], rrow[0:1, :])
                                if qh == 0:
                                    nc.vector.tensor_mul(
                                        aoT[ph][0:64, :],
                                        Us[h - 1][0:64, :], rb[:])
                                else:
                                    tmp = dvp.tile([64, T], BF16,
                                                   tag="tmp", bufs=2,
                                                   name="tmp")
                                    nc.vector.tensor_mul(
                                        tmp[:], Us[h - 1][0:64, :], rb[:])
                                    nc.sync.dma_start(
                                        aoT[ph][64:128, :], tmp[:])
                        divide_head(H - 1)
                  if upto == 3:
                      dump_fm(aoT, ND)
                      raise _Done()

                  # ==== phase 4: projection + residual, LN2 stats fused ====
                  with (
                      tc.tile_pool(name="wpj", bufs=8) as wpjp,
                      tc.tile_pool(name="sq2", bufs=8) as sq2,
                      tc.tile_pool(name="ps_c", bufs=1, space="PSUM") as psc,
                  ):
                      wp_t = []
                      for c in range(ND):
                          w = wpjp.tile([128, D], BF16, tag="wpj")
                          nc.sync.dma_start(
                              w[:], wpj[128 * c:128 * (c + 1), :])
                          wp_t.append(w)
                      st_x = [psc.tile([1, 512], F32, tag="stx", bufs=2,
                                       name="st_x") for _ in range(2)]
                      st_q = [psc.tile([1, 512], F32, tag="stq", bufs=2,
                                       name="st_q") for _ in range(2)]
                      sq = [sq2.tile([128, T], F32R, tag="sq", name="sq")
                            for _ in range(ND)]
                      for co in range(ND):
                          for jj in range(2):
                              sl = slice(512 * jj, 512 * (jj + 1))
                              ps = psc.tile([128, 512], F32, tag="pj",
                                            bufs=2, name="pspj")
                              for c in range(ND):
                                  nc.tensor.matmul(
                                      ps[:],
                                      wp_t[c][:, 128 * co:128 * (co + 1)],
                                      aoT[c][:, sl],
                                      start=(c == 0), stop=(c == ND - 1))
                              nc.vector.scalar_tensor_tensor(
                                  out=xln1[co][:, sl], in0=ps[:],
                                  scalar=c_bp[:, co:co + 1],
                                  in1=xln1[co][:, sl],
                                  op0=ALU.add, op1=ALU.add)
                          nc.vector.tensor_mul(sq[co][:], xln1[co][:],
                                               xln1[co][:])
                          for jj in range(2):
                              sl = slice(512 * jj, 512 * (jj + 1))
                              nc.tensor.matmul(
                                  st_x[jj][:], c_od[:], xln1[co][:, sl],
                                  start=(co == 0), stop=(co == ND - 1))
                              nc.tensor.matmul(
                                  st_q[jj][:], c_od[:], sq[co][:, sl],
                                  start=(co == 0), stop=(co == ND - 1))
                      for jj in range(2):
                          sl = slice(512 * jj, 512 * (jj + 1))
                          nc.vector.tensor_copy(mu_r[:, sl], st_x[jj][:])
                          nc.vector.tensor_copy(ms_r[:, sl], st_q[jj][:])
                  x2 = xln1
                  if upto == 4:
                      dump_fm(x2, ND)
                      raise _Done()

                # ==== phase 5: LN2 rows + apply + xp2/xb2 ====
                with (
                    tc.tile_pool(name="fmp", bufs=8) as fmp,
                    tc.tile_pool(name="xb2p", bufs=1) as xb2p,
                ):
                    xb2 = [xb2p.tile([128, T], BF16, tag="xb2", bufs=8,
                                     name="xb2") for _ in range(ND)]
                    xp2 = [fmp.tile([128, D], F32, tag="xp2", name="xp2")
                           for _ in range(NT)]
                    with (
                        tc.tile_pool(name="rw5", bufs=1) as rw5,
                        tc.tile_pool(name="ps_e", bufs=1, space="PSUM") as pse,
                    ):
                        sd_r = rw5.tile([1, T], F32, tag="sd_r")
                        rc_r = rw5.tile([1, T], F32, tag="rc_r")
                        nc.vector.tensor_mul(sd_r[:], mu_r[:], mu_r[:])
                        nc.vector.tensor_sub(sd_r[:], ms_r[:], sd_r[:])
                        nc.scalar.activation(sd_r[:], sd_r[:], AF.Sqrt,
                                             bias=c_eps[0:1, :])
                        nc.vector.reciprocal_approx_fast(rc_r[:], sd_r[:])
                        mu_b = xb2p.tile([128, T], F32R, tag="mu_b", bufs=1)
                        r_b = xb2p.tile([128, T], F32R, tag="r_b", bufs=1)
                        for jj in range(2):
                            sl = slice(512 * jj, 512 * (jj + 1))
                            pm = pse.tile([128, 512], F32, tag="bc", bufs=2,
                                          name="pm")
                            nc.tensor.matmul(pm[:], v32(c_on[0:1, :]),
                                             mu_r[:, sl])
                            nc.vector.tensor_copy(mu_b[:, sl], pm[:])
                            pr = pse.tile([128, 512], F32, tag="bc", bufs=2,
                                          name="pr")
                            nc.tensor.matmul(pr[:], v32(c_on[0:1, :]),
                                             rc_r[:, sl])
                            nc.vector.tensor_copy(r_b[:, sl], pr[:])
                        for c in range(ND):
                            nc.vector.tensor_sub(x2[c][:], x2[c][:], mu_b[:])
                            nc.vector.tensor_mul(x2[c][:], x2[c][:], r_b[:])
                            nc.vector.tensor_scalar(
                                out=x2[c][:], in0=x2[c][:],
                                scalar1=c_g2[:, c:c + 1],
                                scalar2=c_be2[:, c:c + 1],
                                op0=ALU.mult, op1=ALU.add)
                            nc.vector.tensor_copy(xb2[c][:], x2[c][:])
                        xln2 = x2
                        if upto == 5:
                            dump_fm(xln2, ND)
                            raise _Done()

                        # token-major xln2 (+b2) residual ledger
                        for m in range(NT):
                            for c in range(ND):
                                sl = slice(128 * c, 128 * (c + 1))
                                pt = pse.tile([128, 128], F32R, tag="tr3",
                                              bufs=2, name="pt3")
                                nc.tensor.transpose(
                                    pt[:], xln2[c][:, 128 * m:128 * (m + 1)],
                                    c_id[:])
                                nc.vector.tensor_add(xp2[m][:, sl], pt[:],
                                                     c_b2[:, sl])

                    # ====== phase 6: FFN (bf16, w1 rotating prefetch) ======
                    with (
                        tc.tile_pool(name="hj", bufs=16) as hjp,
                        tc.tile_pool(name="w2t", bufs=16) as w2p,
                        tc.tile_pool(name="ps_d", bufs=1,
                                     space="PSUM") as psd,
                    ):
                        hjs = [[hjp.tile([128, T], BF16, tag="hj",
                                         name="hj") for _ in range(8)]
                               for _ in range(2)]
                        w2s = [[w2p.tile([128, D], BF16, tag="w2",
                                         name="w2t") for _ in range(8)]
                               for _ in range(2)]
                        for jg in range(4):
                            hj = hjs[jg % 2]
                            w2_t = w2s[jg % 2]
                            for j8 in range(8):
                                j = 8 * jg + j8
                                w1c = w1q[j]
                                f1 = psd.tile([128, T], F32, tag="f1",
                                              bufs=2, name="f1")
                                for jj in range(2):
                                    sl = slice(512 * jj, 512 * (jj + 1))
                                    for c in range(ND):
                                        nc.tensor.matmul(
                                            f1[:, sl],
                                            w1c[:, 128 * c:128 * (c + 1)],
                                            xb2[c][:, sl],
                                            start=(c == 0),
                                            stop=(c == ND - 1))
                                nc.scalar.activation(
                                    hj[j8][:], f1[:], AF.Relu,
                                    bias=c_b1[:, j:j + 1])
                                if j + 8 < NJ:
                                    t = w1sp.tile([128, D], BF16, tag="w1",
                                                  name="w1t")
                                    nc.sync.dma_start(
                                        t[:], w1r[j + 8].rearrange(
                                            "a b c -> a (b c)"))
                                    w1q.append(t)
                                nc.sync.dma_start(
                                    w2_t[j8][:],
                                    w2b[128 * j:128 * (j + 1), :])
                            for m in range(NT):
                                for nb in range(2):
                                    sl = slice(512 * nb, 512 * (nb + 1))
                                    fb = psd.tile([128, 512], F32,
                                                  tag="fb", bufs=4,
                                                  name="fb")
                                    for j8 in range(8):
                                        nc.tensor.matmul(
                                            fb[:],
                                            hj[j8][:,
                                                   128 * m:128 * (m + 1)],
                                            w2_t[j8][:, sl],
                                            start=(j8 == 0),
                                            stop=(j8 == 7))
                                    nc.vector.tensor_add(
                                        xp2[m][:, sl], fb[:],
                                        xp2[m][:, sl])
                        for m in range(NT):
                            nc.sync.dma_start(
                                out_l[128 * m:128 * (m + 1), :],
                                xp2[m][:])
          except _Done:
            pass

    nc.compile()
    return nc


_NC = None


def _get_nc():
    global _NC
    if _NC is None:
        _NC = _build()
    return _NC


def _prep_common(wq, wk, wv, w_proj, b_proj, w1, b1, w2, b2, g1, be1, g2, be2):
    f = np.float32
    bf = ml_dtypes.bfloat16
    wq = np.asarray(wq, f)
    wk = np.asarray(wk, f)
    wv = np.asarray(wv, f)

    def pack_pairs(w):
        # [H, D, E] -> [H/2, 128(dd), ND(c), 128(sub,e)]
        w5 = w.reshape(H // 2, 2, ND, 128, E)
        return np.ascontiguousarray(
            w5.transpose(0, 3, 2, 1, 4).reshape(H // 2, 128, ND, 128))

    w1 = np.asarray(w1, f)
    return {
        "wqp": pack_pairs(wq),
        "wkp": pack_pairs(wk),
        "wva": np.ascontiguousarray(wv.transpose(1, 0, 2).reshape(D, D)),
        "wpj": np.ascontiguousarray(np.asarray(w_proj, f).astype(bf)),
        "w1r": np.ascontiguousarray(
            w1.reshape(ND, 128, NJ, 128).transpose(2, 1, 0, 3).astype(bf)),
        "w2b": np.ascontiguousarray(np.asarray(w2, f).astype(bf)),
        "g1f": np.ascontiguousarray(np.asarray(g1, f).reshape(ND, 128).T),
        "be1f": np.ascontiguousarray(np.asarray(be1, f).reshape(ND, 128).T),
        "g2f": np.ascontiguousarray(np.asarray(g2, f).reshape(ND, 128).T),
        "be2f": np.ascontiguousarray(np.asarray(be2, f).reshape(ND, 128).T),
        "bpf": np.ascontiguousarray(np.asarray(b_proj, f).reshape(ND, 128).T),
        "b1f": np.ascontiguousarray(np.asarray(b1, f).reshape(NJ, 128).T),
        "b2b": np.ascontiguousarray(np.tile(np.asarray(b2, f), (128, 1))),
        "idn": np.eye(128, dtype=f),
        "onz": np.ones((128, 128), f),
        "ond": np.full((128, 1), 1.0 / D, f),
        "mby": np.where(np.arange(128)[None, :] >= np.arange(128)[:, None],
                        1.0, 0.0).astype(bf),
    }


def kernel(x, wq, wk, wv, w_proj, b_proj, w1, b1, w2, b2, g1, be1, g2, be2,
           **bench):
    nc = _get_nc()
    common = _prep_common(wq, wk, wv, w_proj, b_proj, w1, b1, w2, b2,
                          g1, be1, g2, be2)
    x = np.asarray(x, np.float32)
    in_maps = [dict(common, x_l=np.ascontiguousarray(x[b]))
               for b in range(NCORE)]
    res = bass_utils.run_bass_kernel_spmd(
        nc, in_maps, core_ids=list(range(NCORE)), **bench)
    out = np.stack([res.results[b]["out_l"] for b in range(NCORE)])
    if bench:
        kernel.last_results = res
    return out


if __name__ == "__main__":
    _build()
    print("built ok")



# revision 5
# speedup vs baseline: 1.2918x; 1.2918x over previous
"""Trainium2 Bass kernel for a transformer block (LN1->MHA->+res->LN2->FFN->+res).

Sharding: data-parallel over batch. B=8 batch elements == 8 NeuronCores; each
core runs the whole block for one batch element (no collectives).

Per-core dataflow (T=1024, D=1024, 16 heads x 64), tuned for PE clock (HAM)
warmth -- the PE runs at 1.2 GHz until it has been busy for a full ~3.4us
window, then 2.4 GHz; every phase keeps a dense back-to-back matmul stream:
  - ~9 dummy warmup matmuls at kernel start flip the HAM throttle to 8/8
    before the first real QKV matmul
  - LN1 token-major per 128-token chunk (ACT accum_out stats); gamma/beta
    folded into the transpose-evacuation tensor_scalar; activations then
    FEATURE-major [D, T] (f32r) through the dense chain
  - causal attention per head: scoresT[s,t] block-rows land in a single
    2-bank PSUM tile [128,1024]; ONE Exp ACT per (head, s-block) covers the
    whole causal range (fuses 1/8 scale + f32->bf16); score matmuls are
    causally tightened; diagonal masked post-exp on GpSimd
  - softmax denominator from a ones-column in v (attn@[v|1]); per-partition
    reciprocal + tensor_scalar divide on DVE
  - projection in bf16 (weights streamed bf16); LN2 stats (ones-matmuls)
    interleaved with the projection matmuls, row sums copied to SBUF at proj
    end; LN2 row ops + per-feature apply feed both an f32 token-major
    residual ledger (xp2, +b2) and a bf16 copy (xb2) for ff1
  - FFN: w1/w2 streamed bf16 (w1 first set prefetched during attention,
    rotating through a 10-deep pool); relu+bias evac on the (idle) Scalar
    engine; ff2 partial sums accumulate straight into xp2 on DVE
"""

import sys

sys.path.insert(0, "/opt/trn_rl_repo")

import numpy as np
import ml_dtypes

import concourse.bacc as bacc
import concourse.mybir as mybir
from concourse.tile import TileContext
from concourse import bass_utils

F32 = mybir.dt.float32
F32R = mybir.dt.float32r
BF16 = mybir.dt.bfloat16
AF = mybir.ActivationFunctionType
ALU = mybir.AluOpType

B, T, D = 8, 1024, 1024
H, E = 16, 64
DFF = 4 * D
NCORE = 8
NT = T // 128
ND = D // 128
NJ = DFF // 128
LN_EPS = 1e-5


class _Done(Exception):
    pass


def r32(ap):
    return ap.bitcast(F32R)


def v32(ap):
    return ap.bitcast(F32)


def _build(upto=9):
    nc = bacc.Bacc("TRN2", target_bir_lowering=False, debug=False,
                   num_devices=NCORE)

    x_l = nc.dram_tensor("x_l", [T, D], F32R, kind="ExternalInput")
    wqp = nc.dram_tensor("wqp", [H // 2, 128, ND, 128], F32R,
                         kind="ExternalInput")
    wkp = nc.dram_tensor("wkp", [H // 2, 128, ND, 128], F32R,
                         kind="ExternalInput")
    wva = nc.dram_tensor("wva", [D, D], F32R, kind="ExternalInput")
    wpj = nc.dram_tensor("wpj", [D, D], BF16, kind="ExternalInput")
    w1r = nc.dram_tensor("w1r", [NJ, 128, ND, 128], BF16,
                         kind="ExternalInput")
    w2b = nc.dram_tensor("w2b", [DFF, D], BF16, kind="ExternalInput")
    g1f = nc.dram_tensor("g1f", [128, ND], F32, kind="ExternalInput")
    be1f = nc.dram_tensor("be1f", [128, ND], F32, kind="ExternalInput")
    g2f = nc.dram_tensor("g2f", [128, ND], F32, kind="ExternalInput")
    be2f = nc.dram_tensor("be2f", [128, ND], F32, kind="ExternalInput")
    bpf = nc.dram_tensor("bpf", [128, ND], F32, kind="ExternalInput")
    b1f = nc.dram_tensor("b1f", [128, NJ], F32, kind="ExternalInput")
    b2b = nc.dram_tensor("b2b", [128, D], F32, kind="ExternalInput")
    idn = nc.dram_tensor("idn", [128, 128], F32R, kind="ExternalInput")
    onz = nc.dram_tensor("onz", [128, 128], F32R, kind="ExternalInput")
    ond = nc.dram_tensor("ond", [128, 1], F32R, kind="ExternalInput")
    mby = nc.dram_tensor("mby", [128, 128], BF16, kind="ExternalInput")
    out_l = nc.dram_tensor("out_l", [T, D], F32, kind="ExternalOutput")

    def dump_fm(tiles, n):
        for c in range(n):
            nc.sync.dma_start(out_l[128 * c:128 * (c + 1), :],
                              v32(tiles[c][:]))

    with TileContext(nc) as tc:
        with tc.tile_pool(name="const", bufs=1) as cp:
          try:
            def cload(name, dram, shape, dtype=F32):
                t = cp.tile(list(shape), dtype, tag=name, name=name)
                nc.sync.dma_start(t[:], dram[:])
                return t

            c_id = cload("idn", idn, [128, 128], F32R)
            c_g1 = cload("g1f", g1f, [128, ND])
            c_be1 = cload("be1f", be1f, [128, ND])
            c_eps = cp.tile([128, 1], F32, tag="eps", name="eps")
            nc.gpsimd.memset(c_eps[:], LN_EPS)
            c_wrm = cp.tile([128, 512], F32, tag="wrm", name="wrm")
            nc.gpsimd.memset(c_wrm[:], 0.0)

            # ---- HAM warmup: ~4us of dummy matmuls so the PE clock is at
            # 2.4 GHz when the first real matmul issues ----
            with tc.tile_pool(name="ps_wrm", bufs=1, space="PSUM") as psw:
                for _ in range(18):
                    pw = psw.tile([128, 512], F32, tag="wrm", bufs=2,
                                  name="pwrm")
                    nc.tensor.matmul(pw[:], c_id[:], r32(c_wrm[:]),
                                     start=True, stop=True)

            with (
                tc.tile_pool(name="w1s", bufs=10) as w1sp,
                tc.tile_pool(name="fmx", bufs=8) as fmx,
                tc.tile_pool(name="rw2", bufs=1) as rw2,
            ):
                # ========== phase 1: LN1 (token-major) ==========
                xf = [fmx.tile([128, T], F32R, tag="xf", name="xf")
                      for _ in range(ND)]
                with (
                    tc.tile_pool(name="xin", bufs=3) as xp,
                    tc.tile_pool(name="scr", bufs=2) as scrp,
                    tc.tile_pool(name="st1", bufs=4) as st1,
                    tc.tile_pool(name="ps_a", bufs=1, space="PSUM") as psa,
                ):
                    for m in range(NT):
                        xm = xp.tile([128, D], F32R, tag="xin")
                        nc.sync.dma_start(xm[:], x_l[128 * m:128 * (m + 1), :])
                        scr = scrp.tile([128, D], F32, tag="scr", name="scr")
                        st = st1.tile([128, 4], F32, tag="st", name="st")
                        nc.scalar.activation(scr[:], xm[:], AF.Square,
                                             accum_out=st[:, 1:2])
                        nc.scalar.activation(scr[:], xm[:], AF.Identity,
                                             accum_out=st[:, 0:1])
                        nc.vector.tensor_scalar_mul(st[:, 0:1], st[:, 0:1],
                                                    1.0 / D)
                        nc.vector.tensor_scalar_mul(st[:, 1:2], st[:, 1:2],
                                                    1.0 / D)
                        nc.vector.tensor_mul(st[:, 2:3], st[:, 0:1],
                                             st[:, 0:1])
                        nc.vector.tensor_sub(st[:, 2:3], st[:, 1:2],
                                             st[:, 2:3])
                        nc.scalar.activation(st[:, 2:3], st[:, 2:3], AF.Sqrt,
                                             bias=c_eps[:])
                        nc.vector.reciprocal(st[:, 3:4], st[:, 2:3])
                        nc.vector.tensor_scalar(
                            out=xm[:], in0=xm[:], scalar1=st[:, 0:1],
                            scalar2=st[:, 3:4], op0=ALU.subtract, op1=ALU.mult)
                        for c in range(ND):
                            pt = psa.tile([128, 128], F32R, tag="tr",
                                          bufs=3, name="pt")
                            nc.tensor.transpose(
                                pt[:], xm[:, 128 * c:128 * (c + 1)], c_id[:])
                            nc.vector.tensor_scalar(
                                out=xf[c][:, 128 * m:128 * (m + 1)],
                                in0=pt[:], scalar1=c_g1[:, c:c + 1],
                                scalar2=c_be1[:, c:c + 1],
                                op0=ALU.mult, op1=ALU.add)
                # non-critical consts load behind the x chunks
                c_idb = cp.tile([128, 128], BF16, tag="idnb", name="idnb")
                nc.vector.tensor_copy(c_idb[:], c_id[:])
                c_on = cload("onz", onz, [128, 128], F32R)
                c_od = cload("ond", ond, [128, 1], F32R)
                c_g2 = cload("g2f", g2f, [128, ND])
                c_be2 = cload("be2f", be2f, [128, ND])
                c_bp = cload("bpf", bpf, [128, ND])
                c_b1 = cload("b1f", b1f, [128, NJ])
                c_b2 = cload("b2b", b2b, [128, D])
                c_mb = cload("mby", mby, [128, 128], BF16)
                if upto == 1:
                    dump_fm(xf, ND)
                    raise _Done()
                xln1 = xf

                # prefetch FFN w1 set 0 (runs during attention on idle DMA)
                w1q = []
                for j8 in range(8):
                    t = w1sp.tile([128, D], BF16, tag="w1", name="w1t")
                    nc.sync.dma_start(
                        t[:], w1r[j8].rearrange("a b c -> a (b c)"))
                    w1q.append(t)

                # LN2 row accumulators (written at proj end, read in phase 5)
                mu_r = rw2.tile([1, T], F32, tag="mu_r")
                ms_r = rw2.tile([1, T], F32, tag="ms_r")

                with tc.tile_pool(name="fma", bufs=8) as fma:
                  aoT = [fma.tile([128, T], BF16, tag="ao", name="aoT")
                         for _ in range(ND)]
                  with (
                      tc.tile_pool(name="qk", bufs=16) as qkp,
                      tc.tile_pool(name="vsb", bufs=8) as vp,
                  ):
                    # ========== phase 2: QKV + v ==========
                    qT, kT, v_sb = [], [], []
                    with (
                        tc.tile_pool(name="wq", bufs=2) as wqpool,
                        tc.tile_pool(name="wk", bufs=2) as wkpool,
                        tc.tile_pool(name="ps_q", bufs=1, space="PSUM") as psq,
                    ):
                        for p in range(H // 2):
                            wq_t = wqpool.tile([128, D], F32R, tag="wq")
                            nc.sync.dma_start(
                                wq_t[:], wqp[p].rearrange("a b c -> a (b c)"))
                            wk_t = wkpool.tile([128, D], F32R, tag="wk")
                            nc.sync.dma_start(
                                wk_t[:], wkp[p].rearrange("a b c -> a (b c)"))
                            q_t = qkp.tile([128, T], BF16, tag="qk")
                            k_t = qkp.tile([128, T], BF16, tag="qk")
                            for dst, w_t in ((q_t, wq_t), (k_t, wk_t)):
                                for jj in range(2):
                                    sl = slice(512 * jj, 512 * (jj + 1))
                                    ps = psq.tile([128, 512], F32, tag="qk",
                                                  bufs=3, name="psqk")
                                    for c in range(ND):
                                        nc.tensor.matmul(
                                            ps[:],
                                            w_t[:, 128 * c:128 * (c + 1)],
                                            xln1[c][:, sl],
                                            start=(c == 0),
                                            stop=(c == ND - 1))
                                    nc.vector.tensor_copy(dst[:, sl], ps[:])
                            qT.append(q_t)
                            kT.append(k_t)
                        with tc.tile_pool(name="wv", bufs=8) as wvpool:
                            wv_t = []
                            for c in range(ND):
                                w = wvpool.tile([128, D], F32R, tag="wv",
                                                name="wv_t")
                                nc.sync.dma_start(
                                    w[:], wva[128 * c:128 * (c + 1), :])
                                wv_t.append(w)
                            # v tiles [128, H, 128]: per head, 64 v-columns
                            # and 64 ones-columns (ones first for odd heads)
                            # -- the av matmul then emits the softmax
                            # denominator broadcast across 64 partitions
                            # right next to the 64 output features.
                            for i in range(NT):
                                vt = vp.tile([128, H * 128], BF16, tag="v",
                                             name="vt")
                                v3 = vt.rearrange("p (h e) -> p h e", e=128)
                                nc.gpsimd.memset(vt[:], 1.0)
                                for nb in range(2):
                                    ps = psq.tile([128, 512], F32, tag="v",
                                                  bufs=2, name="psv")
                                    for c in range(ND):
                                        nc.tensor.matmul(
                                            ps[:],
                                            xln1[c][:, 128 * i:128 * (i + 1)],
                                            wv_t[c][:, 512 * nb:512 * (nb + 1)],
                                            start=(c == 0), stop=(c == ND - 1))
                                    p4 = ps[:].rearrange("p (h e) -> p h e",
                                                         e=64)
                                    nc.scalar.copy(
                                        v3[:, 8 * nb:8 * (nb + 1):2, 0:64],
                                        p4[:, 0:8:2, :])
                                    nc.scalar.copy(
                                        v3[:, 8 * nb + 1:8 * (nb + 1):2,
                                           64:128],
                                        p4[:, 1:8:2, :])
                                v_sb.append(vt)
                    if upto == 2:
                        for c in range(2):
                            nc.sync.dma_start(
                                out_l[128 * c:128 * (c + 1), :],
                                v32(qT[c][:]))
                            nc.sync.dma_start(
                                out_l[128 * (c + 2):128 * (c + 3), :],
                                v32(kT[c][:]))
                        raise _Done()

                    # ===== phase 3: causal attention (v-stationary av) =====
                    # av flipped: out[e,t] = v^T @ se accumulated over key
                    # blocks i straight into a [65,T] psum (row 64 = softmax
                    # denominator via the ones-column in v).  One matmul per
                    # (i, psum bank) instead of per (i, token block): large
                    # moving operands, and the output lands feature-major so
                    # the proj transposes disappear.  Odd heads reach
                    # partitions 64:128 of the pair tile via SBUF->SBUF DMA
                    # (DVE cannot shift partitions).
                    with (
                        tc.tile_pool(name="sc", bufs=12) as scp,
                        tc.tile_pool(name="dvt", bufs=1) as dvp,
                        tc.tile_pool(name="ps_b", bufs=1, space="PSUM") as psb,
                    ):
                        v4i = [v_sb[i].rearrange("p (h e) -> p h e", e=128)
                               for i in range(NT)]
                        Us = [None] * H

                        def div_front(h):
                            # recip of the 64 broadcast denominator copies
                            # (lanes 64(1-q)..) then DMA the recips across
                            # the lane boundary to the feature lanes
                            q = h % 2
                            fsl = slice(64 * q, 64 * (q + 1))
                            dsl = slice(64 * (1 - q), 64 * (2 - q))
                            R = dvp.tile([128, T], F32, tag="R", bufs=2,
                                         name="R")
                            nc.vector.reciprocal(R[dsl, :], Us[h][dsl, :])
                            nc.sync.dma_start(R[fsl, :], R[dsl, :])
                            return R

                        def div_back(h, R):
                            p, q = h // 2, h % 2
                            fsl = slice(64 * q, 64 * (q + 1))
                            nc.vector.tensor_mul(
                                aoT[p][fsl, :], Us[h][fsl, :], R[fsl, :])

                        Rprev = None
                        for h in range(H):
                            p, q = h // 2, h % 2
                            qsl = slice(64 * q, 64 * (q + 1))
                            if h > 0:
                                Rprev = div_front(h - 1)
                            se = []
                            for i in range(NT):
                                st = scp.tile([128, T], BF16, tag="sc",
                                              name="se")
                                sc2 = psb.tile([128, T], F32, tag="sc2",
                                               bufs=2, name="sc2")
                                kblk = kT[p][qsl, 128 * i:128 * (i + 1)]
                                if i < 4:
                                    nc.tensor.matmul(
                                        sc2[:, 128 * i:512], kblk,
                                        qT[p][qsl, 128 * i:512],
                                        start=True, stop=True)
                                lo2 = max(512, 128 * i)
                                nc.tensor.matmul(
                                    sc2[:, lo2:1024], kblk,
                                    qT[p][qsl, lo2:1024],
                                    start=True, stop=True)
                                nc.scalar.activation(
                                    st[:, 128 * i:1024],
                                    sc2[:, 128 * i:1024],
                                    AF.Exp, scale=0.125)
                                dg = slice(128 * i, 128 * (i + 1))
                                nc.gpsimd.tensor_mul(
                                    st[:, dg], st[:, dg], c_mb[:])
                                se.append(st)
                            U = psb.tile([128, T], F32, tag="U", bufs=2,
                                         name="U")
                            Us[h] = U
                            for i in range(NT):
                                lo = 128 * i
                                if lo < 512:
                                    nc.tensor.matmul(
                                        U[:, lo:512], v4i[i][:, h, :],
                                        se[i][:, lo:512],
                                        start=(i == 0), stop=(i == 3),
                                        skip_group_check=True)
                                lo2 = max(lo, 512)
                                nc.tensor.matmul(
                                    U[:, lo2:1024], v4i[i][:, h, :],
                                    se[i][:, lo2:1024],
                                    start=(i == 0), stop=(i == NT - 1),
                                    skip_group_check=True)
                            if h > 0:
                                div_back(h - 1, Rprev)
                        R = div_front(H - 1)
                        div_back(H - 1, R)
                  if upto == 3:
                      dump_fm(aoT, ND)
                      raise _Done()

                  # ==== phase 4: projection + residual, LN2 stats fused ====
                  with (
                      tc.tile_pool(name="wpj", bufs=8) as wpjp,
                      tc.tile_pool(name="sq2", bufs=8) as sq2,
                      tc.tile_pool(name="ps_c", bufs=1, space="PSUM") as psc,
                  ):
                      wp_t = []
                      for c in range(ND):
                          w = wpjp.tile([128, D], BF16, tag="wpj")
                          nc.sync.dma_start(
                              w[:], wpj[128 * c:128 * (c + 1), :])
                          wp_t.append(w)
                      st_x = [psc.tile([1, 512], F32, tag="stx", bufs=2,
                                       name="st_x") for _ in range(2)]
                      st_q = [psc.tile([1, 512], F32, tag="stq", bufs=2,
                                       name="st_q") for _ in range(2)]
                      sq = [sq2.tile([128, T], F32R, tag="sq", name="sq")
                            for _ in range(ND)]
                      for co in range(ND):
                          for jj in range(2):
                              sl = slice(512 * jj, 512 * (jj + 1))
                              ps = psc.tile([128, 512], F32, tag="pj",
                                            bufs=2, name="pspj")
                              for c in range(ND):
                                  nc.tensor.matmul(
                                      ps[:],
                                      wp_t[c][:, 128 * co:128 * (co + 1)],
                                      aoT[c][:, sl],
                                      start=(c == 0), stop=(c == ND - 1))
                              nc.vector.scalar_tensor_tensor(
                                  out=xln1[co][:, sl], in0=ps[:],
                                  scalar=c_bp[:, co:co + 1],
                                  in1=xln1[co][:, sl],
                                  op0=ALU.add, op1=ALU.add)
                          nc.vector.tensor_mul(sq[co][:], xln1[co][:],
                                               xln1[co][:])
                          for jj in range(2):
                              sl = slice(512 * jj, 512 * (jj + 1))
                              nc.tensor.matmul(
                                  st_x[jj][:], c_od[:], xln1[co][:, sl],
                                  start=(co == 0), stop=(co == ND - 1))
                              nc.tensor.matmul(
                                  st_q[jj][:], c_od[:], sq[co][:, sl],
                                  start=(co == 0), stop=(co == ND - 1))
                      for jj in range(2):
                          sl = slice(512 * jj, 512 * (jj + 1))
                          nc.vector.tensor_copy(mu_r[:, sl], st_x[jj][:])
                          nc.vector.tensor_copy(ms_r[:, sl], st_q[jj][:])
                  x2 = xln1
                  if upto == 4:
                      dump_fm(x2, ND)
                      raise _Done()

                # ==== phase 5: LN2 rows + apply + xp2/xb2 ====
                with (
                    tc.tile_pool(name="fmp", bufs=8) as fmp,
                    tc.tile_pool(name="xb2p", bufs=1) as xb2p,
                ):
                    xb2 = [xb2p.tile([128, T], BF16, tag="xb2", bufs=8,
                                     name="xb2") for _ in range(ND)]
                    xp2 = [fmp.tile([128, D], F32, tag="xp2", name="xp2")
                           for _ in range(NT)]
                    with (
                        tc.tile_pool(name="rw5", bufs=1) as rw5,
                        tc.tile_pool(name="ps_e", bufs=1, space="PSUM") as pse,
                    ):
                        sd_r = rw5.tile([1, T], F32, tag="sd_r")
                        rc_r = rw5.tile([1, T], F32, tag="rc_r")
                        nc.vector.tensor_mul(sd_r[:], mu_r[:], mu_r[:])
                        nc.vector.tensor_sub(sd_r[:], ms_r[:], sd_r[:])
                        nc.scalar.activation(sd_r[:], sd_r[:], AF.Sqrt,
                                             bias=c_eps[0:1, :])
                        nc.vector.reciprocal_approx_fast(rc_r[:], sd_r[:])
                        mu_b = xb2p.tile([128, T], F32R, tag="mu_b", bufs=1)
                        r_b = xb2p.tile([128, T], F32R, tag="r_b", bufs=1)
                        for jj in range(2):
                            sl = slice(512 * jj, 512 * (jj + 1))
                            pm = pse.tile([128, 512], F32, tag="bc", bufs=2,
                                          name="pm")
                            nc.tensor.matmul(pm[:], v32(c_on[0:1, :]),
                                             mu_r[:, sl])
                            nc.vector.tensor_copy(mu_b[:, sl], pm[:])
                            pr = pse.tile([128, 512], F32, tag="bc", bufs=2,
                                          name="pr")
                            nc.tensor.matmul(pr[:], v32(c_on[0:1, :]),
                                             rc_r[:, sl])
                            nc.vector.tensor_copy(r_b[:, sl], pr[:])
                        for c in range(ND):
                            nc.vector.tensor_sub(x2[c][:], x2[c][:], mu_b[:])
                            nc.vector.tensor_mul(x2[c][:], x2[c][:], r_b[:])
                            nc.vector.tensor_scalar(
                                out=x2[c][:], in0=x2[c][:],
                                scalar1=c_g2[:, c:c + 1],
                                scalar2=c_be2[:, c:c + 1],
                                op0=ALU.mult, op1=ALU.add)
                            nc.vector.tensor_copy(xb2[c][:], x2[c][:])
                        xln2 = x2
                        if upto == 5:
                            dump_fm(xln2, ND)
                            raise _Done()

                        # token-major xln2 (+b2) residual ledger
                        for m in range(NT):
                            for c in range(ND):
                                sl = slice(128 * c, 128 * (c + 1))
                                pt = pse.tile([128, 128], F32R, tag="tr3",
                                              bufs=2, name="pt3")
                                nc.tensor.transpose(
                                    pt[:], xln2[c][:, 128 * m:128 * (m + 1)],
                                    c_id[:])
                                nc.vector.tensor_add(xp2[m][:, sl], pt[:],
                                                     c_b2[:, sl])

                    # ====== phase 6: FFN (bf16, w1 rotating prefetch) ======
                    with (
                        tc.tile_pool(name="hj", bufs=16) as hjp,
                        tc.tile_pool(name="w2t", bufs=16) as w2p,
                        tc.tile_pool(name="ps_d", bufs=1,
                                     space="PSUM") as psd,
                    ):
                        hjs = [[hjp.tile([128, T], BF16, tag="hj",
                                         name="hj") for _ in range(8)]
                               for _ in range(2)]
                        w2s = [[w2p.tile([128, D], BF16, tag="w2",
                                         name="w2t") for _ in range(8)]
                               for _ in range(2)]
                        for jg in range(4):
                            hj = hjs[jg % 2]
                            w2_t = w2s[jg % 2]
                            for j8 in range(8):
                                j = 8 * jg + j8
                                w1c = w1q[j]
                                f1 = psd.tile([128, T], F32, tag="f1",
                                              bufs=2, name="f1")
                                for jj in range(2):
                                    sl = slice(512 * jj, 512 * (jj + 1))
                                    for c in range(ND):
                                        nc.tensor.matmul(
                                            f1[:, sl],
                                            w1c[:, 128 * c:128 * (c + 1)],
                                            xb2[c][:, sl],
                                            start=(c == 0),
                                            stop=(c == ND - 1))
                                nc.scalar.activation(
                                    hj[j8][:], f1[:], AF.Relu,
                                    bias=c_b1[:, j:j + 1])
                                if j + 8 < NJ:
                                    t = w1sp.tile([128, D], BF16, tag="w1",
                                                  name="w1t")
                                    nc.sync.dma_start(
                                        t[:], w1r[j + 8].rearrange(
                                            "a b c -> a (b c)"))
                                    w1q.append(t)
                                nc.sync.dma_start(
                                    w2_t[j8][:],
                                    w2b[128 * j:128 * (j + 1), :])
                            for m in range(NT):
                                for nb in range(2):
                                    sl = slice(512 * nb, 512 * (nb + 1))
                                    fb = psd.tile([128, 512], F32,
                                                  tag="fb", bufs=4,
                                                  name="fb")
                                    for j8 in range(8):
                                        nc.tensor.matmul(
                                            fb[:],
                                            hj[j8][:,
                                                   128 * m:128 * (m + 1)],
                                            w2_t[j8][:, sl],
                                            start=(j8 == 0),
                                            stop=(j8 == 7))
                                    nc.vector.tensor_add(
                                        xp2[m][:, sl], fb[:],
                                        xp2[m][:, sl])
                        for m in range(NT):
                            nc.sync.dma_start(
                                out_l[128 * m:128 * (m + 1), :],
                                xp2[m][:])
          except _Done:
            pass

    nc.compile()
    return nc


_NC = None


def _get_nc():
    global _NC
    if _NC is None:
        _NC = _build()
    return _NC


def _prep_common(wq, wk, wv, w_proj, b_proj, w1, b1, w2, b2, g1, be1, g2, be2):
    f = np.float32
    bf = ml_dtypes.bfloat16
    wq = np.asarray(wq, f)
    wk = np.asarray(wk, f)
    wv = np.asarray(wv, f)

    def pack_pairs(w):
        # [H, D, E] -> [H/2, 128(dd), ND(c), 128(sub,e)]
        w5 = w.reshape(H // 2, 2, ND, 128, E)
        return np.ascontiguousarray(
            w5.transpose(0, 3, 2, 1, 4).reshape(H // 2, 128, ND, 128))

    w1 = np.asarray(w1, f)
    return {
        "wqp": pack_pairs(wq),
        "wkp": pack_pairs(wk),
        "wva": np.ascontiguousarray(wv.transpose(1, 0, 2).reshape(D, D)),
        "wpj": np.ascontiguousarray(np.asarray(w_proj, f).astype(bf)),
        "w1r": np.ascontiguousarray(
            w1.reshape(ND, 128, NJ, 128).transpose(2, 1, 0, 3).astype(bf)),
        "w2b": np.ascontiguousarray(np.asarray(w2, f).astype(bf)),
        "g1f": np.ascontiguousarray(np.asarray(g1, f).reshape(ND, 128).T),
        "be1f": np.ascontiguousarray(np.asarray(be1, f).reshape(ND, 128).T),
        "g2f": np.ascontiguousarray(np.asarray(g2, f).reshape(ND, 128).T),
        "be2f": np.ascontiguousarray(np.asarray(be2, f).reshape(ND, 128).T),
        "bpf": np.ascontiguousarray(np.asarray(b_proj, f).reshape(ND, 128).T),
        "b1f": np.ascontiguousarray(np.asarray(b1, f).reshape(NJ, 128).T),
        "b2b": np.ascontiguousarray(np.tile(np.asarray(b2, f), (128, 1))),
        "idn": np.eye(128, dtype=f),
        "onz": np.ones((128, 128), f),
        "ond": np.full((128, 1), 1.0 / D, f),
        "mby": np.where(np.arange(128)[None, :] >= np.arange(128)[:, None],
                        1.0, 0.0).astype(bf),
    }


def kernel(x, wq, wk, wv, w_proj, b_proj, w1, b1, w2, b2, g1, be1, g2, be2,
           **bench):
    nc = _get_nc()
    common = _prep_common(wq, wk, wv, w_proj, b_proj, w1, b1, w2, b2,
                          g1, be1, g2, be2)
    x = np.asarray(x, np.float32)
    in_maps = [dict(common, x_l=np.ascontiguousarray(x[b]))
               for b in range(NCORE)]
    res = bass_utils.run_bass_kernel_spmd(
        nc, in_maps, core_ids=list(range(NCORE)), **bench)
    out = np.stack([res.results[b]["out_l"] for b in range(NCORE)])
    if bench:
        kernel.last_results = res
    return out


if __name__ == "__main__":
    _build()
    print("built ok")

